# revision 14
# baseline (speedup 1.0000x reference)
"""AdaINResBlock1 (HiFi-GAN style) Trainium2 kernel, batch-parallel over 8 NeuronCores.

Layout: channels on partitions (4 groups x 128), time on the free axis.
Convs run as bf16 matmuls accumulating f32 in PSUM; weight-norm scale and
conv bias are fused into the PSUM evictions; instance-norm sums ride the
eviction accumulators; the style affine is fused into per-partition
scale/bias operands of ACT/DVE ops.

Overlap strategy (v3):
- input x streams over three DMA rings (sync/scalar/gpsimd); per-chunk
  stats ride the load (sum on DVE, sumsq on ACT).
- the only ACT table set used is trig_and_small (Sin/Square/Identity/Copy);
  every rsqrt runs on the DVE (bit-hack seed + 2 Newton steps), so no
  ACT_TABLE_LOAD ever lands on the stats critical path.
- snake chunks are emitted JUST-IN-TIME between conv waves, so eviction
  instructions sit early in the ACT/DVE queues and PSUM banks recycle
  while the snake for later chunks is still producing.
- conv waves taper (1,1,2,2,1,1): small first waves restart the PE right
  after the stats barrier, small last waves keep the final eviction burst
  off the next stats chain.
- conv1's per-chunk sum-of-squares reads the PSUM tile concurrently with
  the eviction (wnS^2 is folded into the stats), so the square never
  serializes behind the eviction write.
- weight prep is split: stage DMAs + casts + squares stream in mid-conv,
  the small norm matmuls run right after the conv's matmuls as PE filler
  for the stats/snake bubble.
- the last conv2 streams each evicted chunk straight to DRAM.
"""

import math
import sys
from contextlib import ExitStack
from itertools import cycle

import numpy as np

try:
    import concourse.bass as bass
except ImportError:  # pragma: no cover
    sys.path.insert(0, "/opt/trn_rl_repo")
    import concourse.bass as bass

import concourse.tile as tile
from concourse import bacc, mybir

f32 = mybir.dt.float32
bf16 = mybir.dt.bfloat16
f16 = mybir.dt.float16
i32 = mybir.dt.int32
AF = mybir.ActivationFunctionType
OP = mybir.AluOpType
AX = mybir.AxisListType

B, T_FULL, C, S, KW = 8, 4096, 512, 64, 3
DILATIONS = (1, 3, 5)
EPS = 1e-5
G = C // 128          # 4 channel groups of 128 partitions
PADL = 5              # max dilation -> left/right zero pad for conv1 input
TCH = 512             # t-chunk width (one PSUM bank)
N_CORES = 8


def build_nc(T=T_FULL, max_alpha=1.0, stop_after=None, n_iters=3):
    NT = T // TCH
    # tapered waves: fast restart after the stats barrier, small final
    # eviction burst before the next stats chain
    if NT == 8:
        WAVES = [[0], [1], [2, 3], [4, 5], [6], [7]]
    else:
        WAVES = [list(range(w, min(w + 2, NT))) for w in range(0, NT, 2)]
    # ACT Sin is valid on [-pi, pi] only; each ADD_RANGE_WRAP pass unwraps one
    # period. Bound the angle by max_alpha * 9 (|a| <= (1+gamma)*|xn| + beta
    # stays well under 9 for instance-normalized activations).
    N_WRAPS = max(1, int(math.ceil((max_alpha * 9.0 - math.pi) / (2 * math.pi))))
    PI = math.pi

    nc = bacc.Bacc()
    x_ext = nc.declare_dram_parameter("x", [C, T], f32, isOutput=False)
    s_ext = nc.declare_dram_parameter("s", [S, 1], f32, isOutput=False)
    fc1_w_ext = nc.declare_dram_parameter("fc1_w", [3, S, 2 * C], f32, isOutput=False)
    fc1_b_ext = nc.declare_dram_parameter("fc1_b", [3, 2 * C], f32, isOutput=False)
    alpha1_ext = nc.declare_dram_parameter("alpha1", [3, C], f32, isOutput=False)
    conv1_v_ext = nc.declare_dram_parameter("conv1_v", [3, KW, C, C], f32, isOutput=False)
    conv1_g_ext = nc.declare_dram_parameter("conv1_g", [3, C], f32, isOutput=False)
    conv1_b_ext = nc.declare_dram_parameter("conv1_b", [3, C], f32, isOutput=False)
    fc2_w_ext = nc.declare_dram_parameter("fc2_w", [3, S, 2 * C], f32, isOutput=False)
    fc2_b_ext = nc.declare_dram_parameter("fc2_b", [3, 2 * C], f32, isOutput=False)
    alpha2_ext = nc.declare_dram_parameter("alpha2", [3, C], f32, isOutput=False)
    conv2_v_ext = nc.declare_dram_parameter("conv2_v", [3, KW, C, C], f32, isOutput=False)
    conv2_g_ext = nc.declare_dram_parameter("conv2_g", [3, C], f32, isOutput=False)
    conv2_b_ext = nc.declare_dram_parameter("conv2_b", [3, C], f32, isOutput=False)
    out_ext = nc.declare_dram_parameter("out", [C, T], f32, isOutput=True)

    hw_rr = cycle([0, 1])  # sync / scalar HWDGE rings

    with tile.TileContext(nc) as tc, ExitStack() as ctx:
        persist = ctx.enter_context(tc.tile_pool(name="persist", bufs=1))
        wpool = ctx.enter_context(tc.tile_pool(name="wpool", bufs=1))
        stage = ctx.enter_context(tc.tile_pool(name="stage", bufs=4))
        scr = ctx.enter_context(tc.tile_pool(name="scr", bufs=2))
        small = ctx.enter_context(tc.tile_pool(name="small", bufs=2))
        psc = ctx.enter_context(tc.tile_pool(name="psc", bufs=6, space="PSUM"))
        psm = ctx.enter_context(tc.tile_pool(name="psm", bufs=2, space="PSUM"))

        def hw_eng():
            return (nc.sync, nc.scalar)[next(hw_rr)]

        # ------------- persistent state -------------
        ones_col = persist.tile([128, 1], bf16, name="ones_col")
        nc.gpsimd.memset(ones_col, 1.0)
        ident1 = persist.tile([1, 1], f32, name="ident1")
        nc.gpsimd.memset(ident1, 1.0)
        junk = persist.tile([128, 1], f32, name="junk")
        zero_col = persist.tile([128, 1], f32, name="zero_col")
        nc.gpsimd.memset(zero_col, 0.0)
        eps_col = persist.tile([128, 1], f32, name="eps_col")
        nc.gpsimd.memset(eps_col, EPS)
        # pin the trig_and_small ACT table set (Sin/Square/Identity/Copy all
        # live there) before any real ACT work
        nc.scalar.activation(out=junk, in_=eps_col, func=AF.Sin, bias=zero_col)

        s_sb = persist.tile([S, 1], f32, name="s_sb")
        nc.gpsimd.dma_start(out=s_sb, in_=s_ext[:, :])

        x_cur, b1pad, cb2pad = [], [], []
        for g in range(G):
            xc = persist.tile([128, T], f32, name=f"x_cur_{g}")
            x_cur.append(xc)
            bp = persist.tile([128, PADL + T + PADL], bf16, name=f"b1pad_{g}")
            nc.gpsimd.memset(bp[:, 0:PADL], 0.0)
            nc.gpsimd.memset(bp[:, PADL + T:PADL + T + PADL], 0.0)
            b1pad.append(bp)
            cp = persist.tile([128, 1 + T + 1], bf16, name=f"cb2pad_{g}")
            nc.gpsimd.memset(cp[:, 0:1], 0.0)
            nc.gpsimd.memset(cp[:, 1 + T:1 + T + 1], 0.0)
            cb2pad.append(cp)

        # batched per-channel vector loads: DRAM (3, C) -> (128, 3*G), column
        # i*G + g holds channels g*128..g*128+127 of layer i
        def load_pcvec3(name, ext):
            t = persist.tile([128, 3 * G], f32, name=name)
            nc.gpsimd.dma_start(
                out=t, in_=ext.rearrange("i (g p) -> p (i g)", p=128))
            return t

        def lay(t, i):
            return t[:, i * G:(i + 1) * G]

        NCC = 2 * C // 128

        # ------------- DVE rsqrt (no ACT sqrt -> no table switch) -------------
        def emit_rsqrt(dst, v, tag):
            """dst = 1/sqrt(v), elementwise on [128, n] f32 SBUF tiles.

            Quake-style seed 0x5f3759df - (v>>1), computed as (v>>1)*-1 + magic
            on the int32 view (DVE int add/mult goes through f32, so the value
            must stay within 2^31 and may round a few ULP -- both fine for a
            Newton seed); then two Newton steps -> ~5e-6 rel err."""
            nc.vector.tensor_scalar(
                dst.bitcast(i32), v.bitcast(i32), 1, None,
                OP.logical_shift_right)
            nc.vector.tensor_scalar(
                dst.bitcast(i32), dst.bitcast(i32), -1, 0x5F3759DF,
                OP.mult, OP.add)
            t = small.tile(list(v.shape), f32, tag=f"nr_{tag}", name=f"nr_{tag}")
            for _ in range(2):
                nc.vector.tensor_tensor(out=t, in0=dst, in1=dst, op=OP.mult)
                nc.vector.tensor_tensor(out=t, in0=t, in1=v, op=OP.mult)
                nc.vector.tensor_scalar(t, t, -0.5, 1.5, OP.mult, OP.add)
                nc.vector.tensor_tensor(out=dst, in0=dst, in1=t, op=OP.mult)
            return dst

        sqS_t = {}    # 1/sqrt(alpha), filled in after the pcvec loads
        invA_t = {}   # 1/alpha
        alpha_t, g_t, cb_t, fcb_all = {}, {}, {}, {}

        # ------------- weight prep, split into load + norm phases -------------
        def wprep_load(i, which, rings=None, split=False):
            """Stage DMAs + bf16 casts (gpsimd) + squares (DVE) for conv
            (i, which). Casts/squares sit off the ACT queue so nothing
            blocks behind weight-tile DMA completions. Returns state for
            wprep_norm. With split=True, returns (dma_fn, rest_fn) so the
            DMAs can be emitted early and the compute later."""
            vext = conv1_v_ext if which == 1 else conv2_v_ext
            W = [None] * (KW * G)
            st_vs = [None] * (KW * G)
            vsqs = []

            def dmas():
                rr = cycle(rings) if rings else None
                for ci in range(G):
                    for k in range(KW):
                        idx = k * G + ci
                        st_v = stage.tile([128, TCH], f32, tag="vstg", bufs=6,
                                          name=f"vst_{i}_{which}_{k}_{ci}")
                        eng = next(rr) if rr else hw_eng()
                        eng.dma_start(
                            out=st_v,
                            in_=vext[i, k, ci * 128:(ci + 1) * 128, :])
                        st_vs[idx] = st_v

            def rest():
                for ci in range(G):
                    for k in range(KW):
                        idx = k * G + ci
                        st_v = st_vs[idx]
                        W[idx] = wpool.tile(
                            [128, TCH], bf16, tag=f"w{which}_{idx}",
                            name=f"w{which}_{i}_{idx}")
                        nc.gpsimd.tensor_copy(out=W[idx], in_=st_v)
                        vsq = scr.tile([128, TCH], bf16, tag=f"vsq{idx % 6}",
                                       name=f"vsq_{i}_{which}_{k}_{ci}", bufs=1)
                        nc.vector.tensor_tensor(out=vsq, in0=st_v, in1=st_v,
                                                op=OP.mult)
                        vsqs.append(vsq)

            if split:
                return (W, vsqs), dmas, rest
            dmas()
            rest()
            return W, vsqs

        def wprep_norm(i, which, state):
            """Tiny norm matmuls + DVE rsqrt -> per-co scale g/||v||.
            Emitted right after a conv's matmuls: fills the stats bubble."""
            W, vsqs = state
            normps = psm.tile([1, C], f32, tag="pm", name=f"norm_{i}_{which}")
            for n, vsq in enumerate(vsqs):
                nc.tensor.matmul(
                    normps, ones_col, vsq,
                    start=(n == 0), stop=(n == KW * G - 1))
            nrow = small.tile([1, C], f32, tag="nrow", name=f"nrow_{i}_{which}",
                              bufs=2)
            nc.scalar.activation(out=nrow, in_=normps, func=AF.Copy)
            ps_t = psm.tile([128, G], f32, tag="pm", name=f"wnt_{i}_{which}")
            for g in range(G):
                nc.tensor.matmul(
                    ps_t[:, g:g + 1], nrow[0:1, g * 128:(g + 1) * 128], ident1,
                    is_transpose=True, start=(g == 0), stop=(g == G - 1))
            nsq = small.tile([128, G], f32, tag="nsq", name=f"nsq_{i}_{which}",
                             bufs=2)
            nc.vector.tensor_scalar(nsq, ps_t, 0.0, None, OP.add)
            wnS = small.tile([128, G], f32, tag=f"wns{which}",
                             name=f"wns_{i}_{which}", bufs=2)
            emit_rsqrt(wnS, nsq, f"wn{which}")
            nc.vector.tensor_tensor(out=wnS, in0=wnS, in1=lay(g_t[which], i),
                                    op=OP.mult)
            return W, wnS

        def emit_fc(i, which):
            wext = fc1_w_ext if which == 1 else fc2_w_ext
            fcw = stage.tile([S, 2 * C], f32, tag="fcw", name=f"fcw_{i}_{which}",
                             bufs=2)
            hw_eng().dma_start(out=fcw, in_=wext[i])
            hps = psm.tile([128, NCC], f32, tag="pm", name=f"hps_{i}_{which}")
            for cc in range(NCC):
                nc.tensor.matmul(
                    hps[:, cc:cc + 1],
                    fcw[:, cc * 128:(cc + 1) * 128],
                    s_sb,
                    start=(cc == 0), stop=(cc == NCC - 1))
            h_sb = small.tile([128, NCC], f32, tag="hsb",
                              name=f"h_{i}_{which}", bufs=2)
            nc.vector.tensor_tensor(
                out=h_sb, in0=hps,
                in1=fcb_all[which][:, i * NCC:(i + 1) * NCC], op=OP.add)
            return h_sb

        def emit_stats(tag, sum3d, sq3d, h_sb, alpha, sq_scale=None):
            """AdaIN coefficients from per-chunk sums: returns A, B, sinS, sinB.

            Pure-DVE chain (incl. rsqrt) so nothing waits on ACT tables.
            sq_scale: optional per-(partition,group) factor applied to the
            reduced sum of squares (wnS^2 when squares were taken pre-scale
            from PSUM)."""
            sums = small.tile([128, G], f32, tag="sums", name=f"sums_{tag}")
            nc.vector.tensor_reduce(sums, sum3d, axis=AX.X, op=OP.add)
            sqs = small.tile([128, G], f32, tag="sqs", name=f"sqs_{tag}")
            nc.vector.tensor_reduce(sqs, sq3d, axis=AX.X, op=OP.add)
            if sq_scale is not None:
                nc.vector.tensor_tensor(out=sqs, in0=sqs, in1=sq_scale,
                                        op=OP.mult)
            mu = small.tile([128, G], f32, tag="mu", name=f"mu_{tag}")
            nc.vector.tensor_scalar(mu, sums, 1.0 / T, None, OP.mult)
            var = small.tile([128, G], f32, tag="var", name=f"var_{tag}")
            nc.vector.tensor_tensor(out=var, in0=mu, in1=mu, op=OP.mult)
            nc.vector.scalar_tensor_tensor(
                out=var, in0=sqs, scalar=1.0 / T, in1=var,
                op0=OP.mult, op1=OP.subtract)
            nc.vector.tensor_scalar(var, var, EPS, None, OP.add)
            istd = small.tile([128, G], f32, tag="istd", name=f"istd_{tag}")
            emit_rsqrt(istd, var, "istd")
            A = small.tile([128, G], f32, tag="A", name=f"A_{tag}")
            nc.vector.tensor_scalar(A, h_sb[:, 0:G], 1.0, None, OP.add)  # 1+gamma
            nc.vector.tensor_tensor(out=A, in0=A, in1=istd, op=OP.mult)
            Bc = small.tile([128, G], f32, tag="Bc", name=f"B_{tag}")
            nc.vector.tensor_tensor(out=Bc, in0=mu, in1=A, op=OP.mult)
            nc.vector.tensor_tensor(out=Bc, in0=h_sb[:, G:2 * G], in1=Bc,
                                    op=OP.subtract)
            sinS = small.tile([128, G], f32, tag="sinS", name=f"sinS_{tag}")
            nc.vector.tensor_tensor(out=sinS, in0=A, in1=alpha, op=OP.mult)
            sinB = small.tile([128, G], f32, tag="sinB", name=f"sinB_{tag}")
            nc.vector.tensor_tensor(out=sinB, in0=Bc, in1=alpha, op=OP.mult)
            return A, Bc, sinS, sinB

        def snake_chunk(tag, cj, src_fn, dst_fn, sinS, sinB, sqS, invA):
            """dst = (Ax+B) + sin(alpha*(Ax+B))^2 / alpha for one 512-col
            chunk, all groups, written as (t + sin(wrap(t))^2)/alpha with
            t = alpha*(Ax+B) = sinS*x + sinB.

            ACT Sin is only valid on [-pi, pi], so the angle t is computed
            explicitly (gpsimd), range-wrapped INTO A SEPARATE TILE on the
            DVE, Sin/Square (pre-scaled by 1/sqrt(alpha)) run on ACT, and
            the final combine t/alpha + sin^2/alpha is one all-16-bit DVE
            scalar_tensor_tensor."""
            csl = slice(cj * TCH, (cj + 1) * TCH)
            for g in range(G):
                t_g = scr.tile([128, TCH], f16, tag="ang",
                               name=f"ang_{tag}_{cj}_{g}", bufs=6)
                w_g = scr.tile([128, TCH], f16, tag="wrap",
                               name=f"wrap_{tag}_{cj}_{g}", bufs=6)
                sin_g = scr.tile([128, TCH], f16, tag="sin",
                                 name=f"sin_{tag}_{cj}_{g}", bufs=6)
                nc.gpsimd.tensor_scalar(
                    t_g, src_fn(g)[:, csl],
                    sinS[:, g:g + 1], sinB[:, g:g + 1],
                    OP.mult, OP.add)
                nc.vector.add_range_wrap(w_g, t_g, 0.0, PI, 2.0 * PI)
                for _ in range(N_WRAPS - 1):
                    nc.vector.add_range_wrap(w_g, w_g, 0.0, PI, 2.0 * PI)
                nc.scalar.activation(out=sin_g, in_=w_g,
                                     func=AF.Sin, bias=zero_col)
                nc.scalar.activation(out=sin_g, in_=sin_g,
                                     func=AF.Square,
                                     scale=sqS[:, g:g + 1], bias=zero_col)
                nc.vector.scalar_tensor_tensor(
                    out=dst_fn(g)[:, csl], in0=t_g,
                    scalar=invA[:, g:g + 1], in1=sin_g,
                    op0=OP.mult, op1=OP.add)

        def emit_sq_chunk(src_ap, slot_ap, parity, tag):
            """Accumulate sum(src^2) into slot (src in SBUF)."""
            dst = scr.tile([128, TCH], bf16, tag="sqd", name=f"sqd_{tag}",
                           bufs=4)
            if parity:
                nc.scalar.activation(out=dst, in_=src_ap, func=AF.Square,
                                     bias=zero_col, accum_out=slot_ap)
            else:
                nc.vector.affine_mul_reduce(
                    out=dst, accum_out=slot_ap, in0=src_ap, in1=src_ap,
                    scale=1.0, bias=0.0)

        def emit_conv(tag, W, src_pad, pad, d, evict_fn, post_fn=None,
                      snake_fn=None, inject=None, pre_fn=None):
            """Conv waves with just-in-time snake production and mid-conv
            injection hooks (weight prefetch for the next conv)."""
            produced = 0
            for wi, wave in enumerate(WAVES):
                if snake_fn is not None:
                    need = min(NT, wave[-1] + 2)
                    while produced < need:
                        snake_fn(produced)
                        produced += 1
                if pre_fn is not None:
                    pre_fn(wave)
                for co in range(G):
                    pts = [
                        psc.tile([128, TCH], f32, tag="pc",
                                 name=f"ps_{tag}_{co}_{tj}")
                        for tj in wave
                    ]
                    for ci in range(G):
                        for k in range(KW):
                            first = (ci == 0 and k == 0)
                            last = (ci == G - 1 and k == KW - 1)
                            for pt, tj in zip(pts, wave):
                                off = pad + tj * TCH + (k - 1) * d
                                nc.tensor.matmul(
                                    pt,
                                    W[k * G + ci][:, co * 128:(co + 1) * 128],
                                    src_pad[ci][:, off:off + TCH],
                                    start=first, stop=last)
                    for pt, tj in zip(pts, wave):
                        evict_fn(co, tj, pt)
                        if post_fn is not None:
                            post_fn(co, tj, pt)
                if inject is not None and wi in inject:
                    for fn in inject.pop(wi):
                        fn()
            if snake_fn is not None:
                while produced < NT:
                    snake_fn(produced)
                    produced += 1

        # ------------- input load: x + W1_0 over three rings, stats ride ------
        # Emission order is chosen so no in-order engine queue head-of-line
        # blocks another pipeline: (1) x-chunk DMAs on all three rings,
        # (2) W1 stage DMAs on all three rings, (3) input stats (sum on DVE,
        # square on ACT), (4) W1 casts (gpsimd) + squares (DVE), (5) the
        # small pcvec/fcb DMAs (gpsimd), (6) sqS/invA + layer-0 norm MMs.
        xsum_cur = small.tile([128, G, NT], f32, tag="xsum", name="xsum_in")
        xsq_cur = small.tile([128, G, NT], f32, tag="xsq", name="xsq_in")
        TH = T // 2
        NTH = NT // 2
        rings3 = [nc.sync, nc.scalar, nc.gpsimd]
        x_engs = cycle(rings3)
        for h in range(2):
            for g in range(G):
                hsl = slice(h * TH, (h + 1) * TH)
                next(x_engs).dma_start(
                    out=x_cur[g][:, hsl], in_=x_ext[g * 128:(g + 1) * 128, hsl])

        st1, w1_dmas, w1_rest = wprep_load(0, 1, rings=rings3, split=True)
        w1_dmas()

        for h in range(2):
            for g in range(G):
                for sub in range(NTH):
                    tj = h * NTH + sub
                    csl = slice(tj * TCH, (tj + 1) * TCH)
                    nc.vector.tensor_reduce(
                        xsum_cur[:, g, tj:tj + 1], x_cur[g][:, csl],
                        axis=AX.X, op=OP.add)
                    emit_sq_chunk(x_cur[g][:, csl], xsq_cur[:, g, tj:tj + 1],
                                  parity=1, tag=f"xin_{g}_{tj}")

        w1_rest()

        # small per-channel vectors now (gpsimd ring is free again)
        alpha_t = {1: load_pcvec3("alpha1_all", alpha1_ext),
                   2: load_pcvec3("alpha2_all", alpha2_ext)}
        g_t = {1: load_pcvec3("g1_all", conv1_g_ext),
               2: load_pcvec3("g2_all", conv2_g_ext)}
        cb_t = {1: load_pcvec3("cb1_all", conv1_b_ext),
                2: load_pcvec3("cb2_all", conv2_b_ext)}
        for which, bext in ((1, fc1_b_ext), (2, fc2_b_ext)):
            t = persist.tile([128, 3 * NCC], f32, name=f"fcb{which}_all")
            nc.gpsimd.dma_start(
                out=t, in_=bext.rearrange("i (c p) -> p (i c)", p=128))
            fcb_all[which] = t
        for which in (1, 2):
            allt = persist.tile([128, 3 * G], f32, name=f"sqS{which}_all")
            emit_rsqrt(allt, alpha_t[which][:, 0:3 * G], f"sa{which}")
            sqS_t[which] = allt
            inv = persist.tile([128, 3 * G], f32, name=f"invA{which}_all")
            nc.vector.reciprocal(inv, alpha_t[which][:, 0:3 * G])
            invA_t[which] = inv

        # norm matmuls: cheap PE work during the W-gated trickle at conv1_0
        W1_cur, wnS1_cur = wprep_norm(0, 1, st1)
        W2_cur, wnS2_cur = None, None  # prepped during conv1_0

        # ------------- iterations -------------
        pending_bias = None

        for i in range(n_iters):
            d = DILATIONS[i]
            h1 = emit_fc(i, 1)
            h2 = emit_fc(i, 2)

            A1, B1, sinS1, sinB1 = emit_stats(
                f"a1_{i}", xsum_cur, xsq_cur, h1, lay(alpha_t[1], i))

            def snake1(cj, i=i, sS=sinS1, sB=sinB1):
                snake_chunk(f"s1_{i}", cj,
                            src_fn=lambda g: x_cur[g][:, 0:T],
                            dst_fn=lambda g: b1pad[g][:, PADL:PADL + T],
                            sinS=sS, sinB=sB,
                            sqS=lay(sqS_t[1], i), invA=lay(invA_t[1], i))

            c1sum = small.tile([128, G, NT], f32, tag="c1sum", name=f"c1sum_{i}")
            c1sq = small.tile([128, G, NT], f32, tag="c1sq", name=f"c1sq_{i}")

            # conv1 bias is a per-channel constant absorbed exactly by the
            # following instance norm, so it is not applied at all.
            def evict1(co, tj, pt, c1sum=c1sum, wnS1=wnS1_cur):
                dst = cb2pad[co][:, 1 + tj * TCH: 1 + (tj + 1) * TCH]
                if (tj + co) % 2 == 0:
                    nc.scalar.activation(
                        out=dst, in_=pt, func=AF.Identity,
                        bias=zero_col, scale=wnS1[:, co:co + 1],
                        accum_out=c1sum[:, co, tj:tj + 1])
                else:
                    nc.vector.tensor_scalar(
                        dst, pt, wnS1[:, co:co + 1], None, OP.mult, OP.add,
                        accum_out=c1sum[:, co, tj:tj + 1])

            # squares straight from PSUM (concurrent with the eviction, no
            # RAW on the evicted chunk); wnS^2 folded into the stats
            def post1(co, tj, pt, i=i, c1sq=c1sq):
                dst = scr.tile([128, TCH], bf16, tag="sqd",
                               name=f"sqd_c1_{i}_{co}_{tj}", bufs=4)
                nc.scalar.activation(out=dst, in_=pt, func=AF.Square,
                                     bias=zero_col,
                                     accum_out=c1sq[:, co, tj:tj + 1])

            # prefetch conv2's weights mid-conv1
            inject1 = {}
            wst = {}
            if i == 0:
                inject1[1] = [lambda: wst.__setitem__(2, wprep_load(0, 2))]
            else:
                inject1[1] = [lambda i=i: wst.__setitem__(2, wprep_load(i, 2))]

            emit_conv(f"c1_{i}", W1_cur, b1pad, PADL, d, evict1, post1,
                      snake_fn=snake1, inject=inject1)

            # norm matmuls for conv2's weights: PE filler in the stats bubble
            W2_cur, wnS2_cur = wprep_norm(i, 2, wst[2])

            wnS1sq = small.tile([128, G], f32, tag="wnsq", name=f"wnsq_{i}")
            nc.vector.tensor_tensor(out=wnS1sq, in0=wnS1_cur, in1=wnS1_cur,
                                    op=OP.mult)
            A2, B2, sinS2, sinB2 = emit_stats(
                f"a2_{i}", c1sum, c1sq, h2, lay(alpha_t[2], i),
                sq_scale=wnS1sq)

            def snake2(cj, i=i, sS=sinS2, sB=sinB2):
                snake_chunk(f"s2_{i}", cj,
                            src_fn=lambda g: cb2pad[g][:, 1:1 + T],
                            dst_fn=lambda g: cb2pad[g][:, 1:1 + T],
                            sinS=sS, sinB=sB,
                            sqS=lay(sqS_t[2], i), invA=lay(invA_t[2], i))

            # conv2 bias is a per-channel constant: every downstream consumer
            # of x except the final output is an instance norm (which absorbs
            # it) or the residual chain. Accumulate it and apply per chunk
            # just before the final conv2's evictions.
            if pending_bias is None:
                pending_bias = small.tile([128, G], f32, tag="pend",
                                          name="pending_bias", bufs=1)
                nc.vector.tensor_copy(pending_bias, lay(cb_t[2], i))
            else:
                nc.vector.tensor_tensor(out=pending_bias, in0=pending_bias,
                                        in1=lay(cb_t[2], i), op=OP.add)

            last = (i == n_iters - 1)

            def bias_wave(wave, pending_bias=pending_bias):
                # per-chunk deferred-bias adds for this wave's chunks, emitted
                # after the wave's snake chunks so they don't block the snake
                # Sin ops on the in-order ACT queue
                for co in range(G):
                    for tj in wave:
                        sl = x_cur[co][:, tj * TCH:(tj + 1) * TCH]
                        nc.scalar.activation(
                            out=sl, in_=sl, func=AF.Identity,
                            bias=pending_bias[:, co:co + 1], scale=1.0)

            xsum_nxt = small.tile([128, G, NT], f32, tag="xsum", name=f"xsum_{i}")
            xsq_nxt = small.tile([128, G, NT], f32, tag="xsq", name=f"xsq_{i}")

            def evict2(co, tj, pt, wnS2=wnS2_cur, xsum_nxt=xsum_nxt):
                sl = x_cur[co][:, tj * TCH:(tj + 1) * TCH]
                nc.vector.scalar_tensor_tensor(
                    out=sl, in0=pt, scalar=wnS2[:, co:co + 1], in1=sl,
                    op0=OP.mult, op1=OP.add,
                    accum_out=xsum_nxt[:, co, tj:tj + 1])

            def post2(co, tj, pt, i=i, xsq_nxt=xsq_nxt, last=last):
                sl = x_cur[co][:, tj * TCH:(tj + 1) * TCH]
                if last:
                    # stream the finished chunk straight out
                    hw_eng().dma_start(
                        out=out_ext[co * 128:(co + 1) * 128,
                                    tj * TCH:(tj + 1) * TCH],
                        in_=sl)
                else:
                    emit_sq_chunk(sl, xsq_nxt[:, co, tj:tj + 1],
                                  parity=(tj + co) % 2, tag=f"x_{i}_{co}_{tj}")

            # prefetch next layer's conv1 weights mid-conv2
            inject2 = {}
            if i < n_iters - 1:
                inject2[1] = [lambda i=i: wst.__setitem__(1, wprep_load(i + 1, 1))]

            emit_conv(f"c2_{i}", W2_cur, cb2pad, 1, 1, evict2, post2,
                      snake_fn=snake2, inject=inject2,
                      pre_fn=bias_wave if last else None)
            xsum_cur, xsq_cur = xsum_nxt, xsq_nxt

            if i < n_iters - 1:
                # norm matmuls: PE filler in the next stats bubble
                W1_cur, wnS1_cur = wprep_norm(i + 1, 1, wst[1])

    return nc


def make_in_maps(inputs, T=T_FULL):
    npf = lambda v: np.asarray(v, dtype=np.float32)
    x = npf(inputs["x"])
    s = npf(inputs["s"])
    shared = {
        "fc1_w": npf(inputs["fc1_w"]),
        "fc1_b": npf(inputs["fc1_b"]),
        "alpha1": npf(inputs["alpha1"]).reshape(3, C),
        "conv1_v": npf(inputs["conv1_v"]),
        "conv1_g": npf(inputs["conv1_g"]),
        "conv1_b": npf(inputs["conv1_b"]),
        "fc2_w": npf(inputs["fc2_w"]),
        "fc2_b": npf(inputs["fc2_b"]),
        "alpha2": npf(inputs["alpha2"]).reshape(3, C),
        "conv2_v": npf(inputs["conv2_v"]),
        "conv2_g": npf(inputs["conv2_g"]),
        "conv2_b": npf(inputs["conv2_b"]),
    }
    in_maps = []
    for b in range(N_CORES):
        m = dict(shared)
        m["x"] = np.ascontiguousarray(x[b, :T, :].T)
        m["s"] = np.ascontiguousarray(s[b].reshape(S, 1))
        in_maps.append(m)
    return in_maps


_CACHED = {}


def kernel(**inputs) -> np.ndarray:
    from concourse.bass_utils import run_bass_kernel_spmd

    max_alpha = float(max(np.abs(np.asarray(inputs["alpha1"])).max(),
                          np.abs(np.asarray(inputs["alpha2"])).max()))
    key = ("nc", max_alpha)
    if key not in _CACHED:
        nc = build_nc(T_FULL, max_alpha=max_alpha)
        nc.finalize()
        _CACHED[key] = nc
    nc = _CACHED[key]
    in_maps = make_in_maps(inputs, T_FULL)
    res = run_bass_kernel_spmd(nc, in_maps, core_ids=list(range(N_CORES)))
    out = np.stack(
        [np.asarray(res.results[i]["out"]).T for i in range(N_CORES)], axis=0)
    return np.ascontiguousarray(out).astype(np.float32)


# revision 29
# speedup vs baseline: 1.1154x; 1.1154x over previous
"""AdaINResBlock1 (HiFi-GAN style) Trainium2 kernel, batch-parallel over 8 NeuronCores.

Layout: channels on partitions (4 groups x 128), time on the free axis.
Convs run as bf16 matmuls accumulating f32 in PSUM; weight-norm scale and
conv bias are fused into the PSUM evictions; instance-norm sums ride the
eviction accumulators; the style affine is fused into per-partition
scale/bias operands of ACT/DVE ops.

Overlap strategy (v3):
- input x streams over three DMA rings (sync/scalar/gpsimd); per-chunk
  stats ride the load (sum on DVE, sumsq on ACT).
- the only ACT table set used is trig_and_small (Sin/Square/Identity/Copy);
  every rsqrt runs on the DVE (bit-hack seed + 2 Newton steps), so no
  ACT_TABLE_LOAD ever lands on the stats critical path.
- snake chunks are emitted JUST-IN-TIME between conv waves, so eviction
  instructions sit early in the ACT/DVE queues and PSUM banks recycle
  while the snake for later chunks is still producing.
- conv waves taper (1,1,2,2,1,1): small first waves restart the PE right
  after the stats barrier, small last waves keep the final eviction burst
  off the next stats chain.
- conv1's per-chunk sum-of-squares reads the PSUM tile concurrently with
  the eviction (wnS^2 is folded into the stats), so the square never
  serializes behind the eviction write.
- weight prep is split: stage DMAs + casts + squares stream in mid-conv,
  the small norm matmuls run right after the conv's matmuls as PE filler
  for the stats/snake bubble.
- the last conv2 streams each evicted chunk straight to DRAM.
"""

import math
import sys
from contextlib import ExitStack
from itertools import cycle

import numpy as np

try:
    import concourse.bass as bass
except ImportError:  # pragma: no cover
    sys.path.insert(0, "/opt/trn_rl_repo")
    import concourse.bass as bass

import concourse.tile as tile
from concourse import bacc, mybir

f32 = mybir.dt.float32
bf16 = mybir.dt.bfloat16
f16 = mybir.dt.float16
i32 = mybir.dt.int32
AF = mybir.ActivationFunctionType
OP = mybir.AluOpType
AX = mybir.AxisListType

B, T_FULL, C, S, KW = 8, 4096, 512, 64, 3
DILATIONS = (1, 3, 5)
EPS = 1e-5
G = C // 128          # 4 channel groups of 128 partitions
PADL = 5              # max dilation -> left/right zero pad for conv1 input
TCH = 512             # t-chunk width (one PSUM bank)
N_CORES = 8


def build_nc(T=T_FULL, max_alpha=1.0, stop_after=None, n_iters=3):
    NT = T // TCH
    # tapered waves: fast restart after the stats barrier, small final
    # eviction burst before the next stats chain
    if NT == 8:
        WAVES = [[0], [1], [2, 3], [4, 5], [6], [7]]
    else:
        WAVES = [list(range(w, min(w + 2, NT))) for w in range(0, NT, 2)]
    # ACT Sin is valid on [-pi, pi] only; each ADD_RANGE_WRAP pass unwraps one
    # period. Bound the angle by max_alpha * 9 (|a| <= (1+gamma)*|xn| + beta
    # stays well under 9 for instance-normalized activations).
    N_WRAPS = max(1, int(math.ceil((max_alpha * 9.0 - math.pi) / (2 * math.pi))))
    PI = math.pi

    nc = bacc.Bacc()
    x_ext = nc.declare_dram_parameter("x", [C, T], f32, isOutput=False)
    s_ext = nc.declare_dram_parameter("s", [S, 1], f32, isOutput=False)
    fc1_w_ext = nc.declare_dram_parameter("fc1_w", [3, S, 2 * C], f32, isOutput=False)
    fc1_b_ext = nc.declare_dram_parameter("fc1_b", [3, 2 * C], f32, isOutput=False)
    alpha1_ext = nc.declare_dram_parameter("alpha1", [3, C], f32, isOutput=False)
    conv1_v_ext = nc.declare_dram_parameter("conv1_v", [3, KW, C, C], f32, isOutput=False)
    conv1_g_ext = nc.declare_dram_parameter("conv1_g", [3, C], f32, isOutput=False)
    conv1_b_ext = nc.declare_dram_parameter("conv1_b", [3, C], f32, isOutput=False)
    fc2_w_ext = nc.declare_dram_parameter("fc2_w", [3, S, 2 * C], f32, isOutput=False)
    fc2_b_ext = nc.declare_dram_parameter("fc2_b", [3, 2 * C], f32, isOutput=False)
    alpha2_ext = nc.declare_dram_parameter("alpha2", [3, C], f32, isOutput=False)
    conv2_v_ext = nc.declare_dram_parameter("conv2_v", [3, KW, C, C], f32, isOutput=False)
    conv2_g_ext = nc.declare_dram_parameter("conv2_g", [3, C], f32, isOutput=False)
    conv2_b_ext = nc.declare_dram_parameter("conv2_b", [3, C], f32, isOutput=False)
    out_ext = nc.declare_dram_parameter("out", [C, T], f32, isOutput=True)

    hw_rr = cycle([0, 1])  # sync / scalar HWDGE rings

    with tile.TileContext(nc) as tc, ExitStack() as ctx:
        persist = ctx.enter_context(tc.tile_pool(name="persist", bufs=1))
        wpool = ctx.enter_context(tc.tile_pool(name="wpool", bufs=1))
        stage = ctx.enter_context(tc.tile_pool(name="stage", bufs=4))
        scr = ctx.enter_context(tc.tile_pool(name="scr", bufs=2))
        small = ctx.enter_context(tc.tile_pool(name="small", bufs=2))
        psc = ctx.enter_context(tc.tile_pool(name="psc", bufs=6, space="PSUM"))
        psm = ctx.enter_context(tc.tile_pool(name="psm", bufs=2, space="PSUM"))

        def hw_eng():
            return (nc.sync, nc.scalar)[next(hw_rr)]

        # ------------- persistent state -------------
        ones_col = persist.tile([128, 1], bf16, name="ones_col")
        nc.gpsimd.memset(ones_col, 1.0)
        ident1 = persist.tile([1, 1], f32, name="ident1")
        nc.gpsimd.memset(ident1, 1.0)
        junk = persist.tile([128, 1], f32, name="junk")
        zero_col = persist.tile([128, 1], f32, name="zero_col")
        nc.gpsimd.memset(zero_col, 0.0)
        eps_col = persist.tile([128, 1], f32, name="eps_col")
        nc.gpsimd.memset(eps_col, EPS)
        # pin the trig_and_small ACT table set (Sin/Square/Identity/Copy all
        # live there) before any real ACT work
        nc.scalar.activation(out=junk, in_=eps_col, func=AF.Sin, bias=zero_col)

        s_sb = persist.tile([S, 1], f32, name="s_sb")
        nc.gpsimd.dma_start(out=s_sb, in_=s_ext[:, :])

        x_cur, b1pad, cb2pad = [], [], []
        for g in range(G):
            xc = persist.tile([128, T], f32, name=f"x_cur_{g}")
            x_cur.append(xc)
            bp = persist.tile([128, PADL + T + PADL], bf16, name=f"b1pad_{g}")
            nc.gpsimd.memset(bp[:, 0:PADL], 0.0)
            nc.gpsimd.memset(bp[:, PADL + T:PADL + T + PADL], 0.0)
            b1pad.append(bp)
            cp = persist.tile([128, 1 + T + 1], bf16, name=f"cb2pad_{g}")
            nc.gpsimd.memset(cp[:, 0:1], 0.0)
            nc.gpsimd.memset(cp[:, 1 + T:1 + T + 1], 0.0)
            cb2pad.append(cp)

        # batched per-channel vector loads: DRAM (3, C) -> (128, 3*G), column
        # i*G + g holds channels g*128..g*128+127 of layer i
        def load_pcvec3(name, ext):
            t = persist.tile([128, 3 * G], f32, name=name)
            nc.gpsimd.dma_start(
                out=t, in_=ext.rearrange("i (g p) -> p (i g)", p=128))
            return t

        def lay(t, i):
            return t[:, i * G:(i + 1) * G]

        NCC = 2 * C // 128

        # ------------- DVE rsqrt (no ACT sqrt -> no table switch) -------------
        def emit_rsqrt(dst, v, tag, iters=2):
            """dst = 1/sqrt(v), elementwise on [128, n] f32 SBUF tiles.

            Quake-style seed 0x5f3759df - (v>>1), computed as (v>>1)*-1 + magic
            on the int32 view (DVE int add/mult goes through f32, so the value
            must stay within 2^31 and may round a few ULP -- both fine for a
            Newton seed); then two Newton steps -> ~5e-6 rel err."""
            nc.vector.tensor_scalar(
                dst.bitcast(i32), v.bitcast(i32), 1, None,
                OP.logical_shift_right)
            nc.vector.tensor_scalar(
                dst.bitcast(i32), dst.bitcast(i32), -1, 0x5F3759DF,
                OP.mult, OP.add)
            t = small.tile(list(v.shape), f32, tag=f"nr_{tag}", name=f"nr_{tag}")
            for _ in range(iters):
                nc.vector.tensor_tensor(out=t, in0=dst, in1=dst, op=OP.mult)
                nc.vector.tensor_tensor(out=t, in0=t, in1=v, op=OP.mult)
                nc.vector.tensor_scalar(t, t, -0.5, 1.5, OP.mult, OP.add)
                nc.vector.tensor_tensor(out=dst, in0=dst, in1=t, op=OP.mult)
            return dst

        sqS_t = {}    # 1/sqrt(alpha), filled in after the pcvec loads
        invA_t = {}   # 1/alpha
        alpha_t, g_t, cb_t, fcb_all = {}, {}, {}, {}

        # ------------- weight prep, split into load + norm phases -------------
        # order of (ci, k) pairs matching conv matmul consumption order
        W_ORDER = [(k * G + ci, ci, k) for ci in range(G) for k in range(KW)]

        def wprep_part(i, which, state, lo, hi, rings=None):
            """Emit [stage DMA, DVE cast, DVE square] for weight tiles
            lo..hi-1 of conv (i, which). Parts are spread over conv waves so
            the DVE lumps never delay the just-in-time snake chunks."""
            vext = conv1_v_ext if which == 1 else conv2_v_ext
            W, vsqs = state
            rr = cycle(rings) if rings else None
            for idx, ci, k in W_ORDER[lo:hi]:
                st_v = stage.tile([128, TCH], f32, tag="vstg", bufs=6,
                                  name=f"vst_{i}_{which}_{k}_{ci}")
                eng = next(rr) if rr else hw_eng()
                eng.dma_start(
                    out=st_v, in_=vext[i, k, ci * 128:(ci + 1) * 128, :])
                W[idx] = wpool.tile(
                    [128, TCH], bf16, tag=f"w{which}_{idx}",
                    name=f"w{which}_{i}_{idx}")
                nc.vector.tensor_copy(out=W[idx], in_=st_v)
                vsq = scr.tile([128, TCH], bf16, tag=f"vsq{idx % 4}",
                               name=f"vsq_{i}_{which}_{k}_{ci}", bufs=1)
                nc.vector.tensor_tensor(out=vsq, in0=st_v, in1=st_v,
                                        op=OP.mult)
                vsqs.append(vsq)

        def wprep_state():
            return ([None] * (KW * G), [])

        def wprep_norm(i, which, state):
            """Tiny norm matmuls + DVE rsqrt -> per-co scale g/||v||.
            Emitted right after a conv's matmuls: fills the stats bubble."""
            W, vsqs = state
            normps = psm.tile([1, C], f32, tag="pm", name=f"norm_{i}_{which}")
            for n, vsq in enumerate(vsqs):
                nc.tensor.matmul(
                    normps, ones_col, vsq,
                    start=(n == 0), stop=(n == KW * G - 1))
            nrow = small.tile([1, C], f32, tag="nrow", name=f"nrow_{i}_{which}",
                              bufs=2)
            nc.scalar.activation(out=nrow, in_=normps, func=AF.Copy)
            ps_t = psm.tile([128, G], f32, tag="pm", name=f"wnt_{i}_{which}")
            for g in range(G):
                nc.tensor.matmul(
                    ps_t[:, g:g + 1], nrow[0:1, g * 128:(g + 1) * 128], ident1,
                    is_transpose=True, start=(g == 0), stop=(g == G - 1))
            nsq = small.tile([128, G], f32, tag="nsq", name=f"nsq_{i}_{which}",
                             bufs=2)
            nc.vector.tensor_scalar(nsq, ps_t, 0.0, None, OP.add)
            wnS = small.tile([128, G], f32, tag=f"wns{which}",
                             name=f"wns_{i}_{which}", bufs=2)
            emit_rsqrt(wnS, nsq, f"wn{which}")
            nc.vector.tensor_tensor(out=wnS, in0=wnS, in1=lay(g_t[which], i),
                                    op=OP.mult)
            return W, wnS

        fcw_tiles = {}

        def fc_dma(i, which):
            wext = fc1_w_ext if which == 1 else fc2_w_ext
            fcw = stage.tile([S, 2 * C], f32, tag="fcw", name=f"fcw_{i}_{which}",
                             bufs=2)
            hw_eng().dma_start(out=fcw, in_=wext[i])
            fcw_tiles[(i, which)] = fcw

        def fc_mm(i, which):
            fcw = fcw_tiles.pop((i, which))
            hps = psm.tile([128, NCC], f32, tag="pm", name=f"hps_{i}_{which}")
            for cc in range(NCC):
                nc.tensor.matmul(
                    hps[:, cc:cc + 1],
                    fcw[:, cc * 128:(cc + 1) * 128],
                    s_sb,
                    start=(cc == 0), stop=(cc == NCC - 1))
            h_sb = small.tile([128, NCC], f32, tag="hsb",
                              name=f"h_{i}_{which}", bufs=2)
            nc.vector.tensor_tensor(
                out=h_sb, in0=hps,
                in1=fcb_all[which][:, i * NCC:(i + 1) * NCC], op=OP.add)
            return h_sb

        def emit_coef(tag, h_sb, alpha):
            """Per-layer constants q = alpha*(1+gamma), ab = alpha*beta.
            Emitted as soon as the fc output exists -- off the stats chain."""
            q = small.tile([128, G], f32, tag="q", name=f"q_{tag}")
            nc.vector.scalar_tensor_tensor(
                out=q, in0=h_sb[:, 0:G], scalar=1.0, in1=alpha,
                op0=OP.add, op1=OP.mult)
            ab = small.tile([128, G], f32, tag="ab", name=f"ab_{tag}")
            nc.vector.tensor_tensor(out=ab, in0=h_sb[:, G:2 * G], in1=alpha,
                                    op=OP.mult)
            return q, ab

        def emit_stats(tag, sum3d, sq3d, coef):
            """sinS/sinB from per-chunk sums: the minimal serial DVE chain
            (13 small ops incl. a 1-Newton rsqrt) after the last eviction."""
            q, ab = coef
            sums = small.tile([128, G], f32, tag="sums", name=f"sums_{tag}")
            nc.vector.tensor_reduce(sums, sum3d, axis=AX.X, op=OP.add)
            sqs = small.tile([128, G], f32, tag="sqs", name=f"sqs_{tag}")
            nc.vector.tensor_reduce(sqs, sq3d, axis=AX.X, op=OP.add)
            mu = small.tile([128, G], f32, tag="mu", name=f"mu_{tag}")
            nc.vector.tensor_scalar(mu, sums, 1.0 / T, None, OP.mult)
            nc.vector.tensor_scalar(sqs, sqs, 1.0 / T, EPS, OP.mult, OP.add)
            var = small.tile([128, G], f32, tag="var", name=f"var_{tag}")
            nc.vector.tensor_tensor(out=var, in0=mu, in1=mu, op=OP.mult)
            nc.vector.tensor_tensor(out=var, in0=sqs, in1=var, op=OP.subtract)
            istd = small.tile([128, G], f32, tag="istd", name=f"istd_{tag}")
            emit_rsqrt(istd, var, "istd", iters=1)
            sinS = small.tile([128, G], f32, tag="sinS", name=f"sinS_{tag}")
            nc.vector.tensor_tensor(out=sinS, in0=q, in1=istd, op=OP.mult)
            sinB = small.tile([128, G], f32, tag="sinB", name=f"sinB_{tag}")
            nc.vector.tensor_tensor(out=sinB, in0=mu, in1=sinS, op=OP.mult)
            nc.vector.tensor_tensor(out=sinB, in0=ab, in1=sinB, op=OP.subtract)
            return sinS, sinB

        def snake_chunk(tag, cj, src_fn, dst_fn, sinS, sinB, sqS, invA):
            """dst = (Ax+B) + sin(alpha*(Ax+B))^2 / alpha for one 512-col
            chunk, all groups, written as (t + sin(wrap(t))^2)/alpha with
            t = alpha*(Ax+B) = sinS*x + sinB.

            ACT Sin is only valid on [-pi, pi], so the angle t is computed
            explicitly (gpsimd), range-wrapped INTO A SEPARATE TILE on the
            DVE, Sin/Square (pre-scaled by 1/sqrt(alpha)) run on ACT, and
            the final combine t/alpha + sin^2/alpha is one all-16-bit DVE
            scalar_tensor_tensor."""
            csl = slice(cj * TCH, (cj + 1) * TCH)
            for g in range(G):
                t_g = scr.tile([128, TCH], f16, tag="ang",
                               name=f"ang_{tag}_{cj}_{g}", bufs=5)
                w_g = scr.tile([128, TCH], f16, tag="wrap",
                               name=f"wrap_{tag}_{cj}_{g}", bufs=5)
                sin_g = scr.tile([128, TCH], f16, tag="sin",
                                 name=f"sin_{tag}_{cj}_{g}", bufs=5)
                nc.gpsimd.tensor_scalar(
                    t_g, src_fn(g)[:, csl],
                    sinS[:, g:g + 1], sinB[:, g:g + 1],
                    OP.mult, OP.add)
                nc.vector.add_range_wrap(w_g, t_g, 0.0, PI, 2.0 * PI)
                for _ in range(N_WRAPS - 1):
                    nc.vector.add_range_wrap(w_g, w_g, 0.0, PI, 2.0 * PI)
                nc.scalar.activation(out=sin_g, in_=w_g,
                                     func=AF.Sin, bias=zero_col)
                nc.scalar.activation(out=sin_g, in_=sin_g,
                                     func=AF.Square,
                                     scale=sqS[:, g:g + 1], bias=zero_col)
                nc.vector.scalar_tensor_tensor(
                    out=dst_fn(g)[:, csl], in0=t_g,
                    scalar=invA[:, g:g + 1], in1=sin_g,
                    op0=OP.mult, op1=OP.add)

        def emit_sq_chunk(src_ap, slot_ap, parity, tag):
            """Accumulate sum(src^2) into slot (src in SBUF)."""
            dst = scr.tile([128, TCH], bf16, tag="sqd", name=f"sqd_{tag}",
                           bufs=3)
            if parity:
                nc.scalar.activation(out=dst, in_=src_ap, func=AF.Square,
                                     bias=zero_col, accum_out=slot_ap)
            else:
                nc.vector.affine_mul_reduce(
                    out=dst, accum_out=slot_ap, in0=src_ap, in1=src_ap,
                    scale=1.0, bias=0.0)

        def emit_conv(tag, W, src_pad, pad, d, evict_fn, post_fn=None,
                      snake_fn=None, inject=None, pre_fn=None):
            """Conv waves with just-in-time snake production and mid-conv
            injection hooks (weight prefetch for the next conv)."""
            produced = 0
            for wi, wave in enumerate(WAVES):
                if snake_fn is not None:
                    need = min(NT, wave[-1] + 2)
                    while produced < need:
                        snake_fn(produced)
                        produced += 1
                if pre_fn is not None:
                    pre_fn(wave)
                for co in range(G):
                    pts = [
                        psc.tile([128, TCH], f32, tag="pc",
                                 name=f"ps_{tag}_{co}_{tj}")
                        for tj in wave
                    ]
                    for ci in range(G):
                        for k in range(KW):
                            first = (ci == 0 and k == 0)
                            last = (ci == G - 1 and k == KW - 1)
                            for pt, tj in zip(pts, wave):
                                off = pad + tj * TCH + (k - 1) * d
                                nc.tensor.matmul(
                                    pt,
                                    W[k * G + ci][:, co * 128:(co + 1) * 128],
                                    src_pad[ci][:, off:off + TCH],
                                    start=first, stop=last)
                    for pt, tj in zip(pts, wave):
                        evict_fn(co, tj, pt)
                        if post_fn is not None:
                            post_fn(co, tj, pt)
                if inject is not None and wi in inject:
                    for fn in inject.pop(wi):
                        fn()
            if snake_fn is not None:
                while produced < NT:
                    snake_fn(produced)
                    produced += 1

        # ------------- input load: x + W1_0 over three rings, stats ride ------
        # Emission order is chosen so no in-order engine queue head-of-line
        # blocks another pipeline: (1) x-chunk DMAs on all three rings,
        # (2) W1 stage DMAs on all three rings, (3) input stats (sum on DVE,
        # square on ACT), (4) W1 casts (gpsimd) + squares (DVE), (5) the
        # small pcvec/fcb DMAs (gpsimd), (6) sqS/invA + layer-0 norm MMs.
        xsum_cur = small.tile([128, G, NT], f32, tag="xsum", name="xsum_in")
        xsq_cur = small.tile([128, G, NT], f32, tag="xsq", name="xsq_in")
        TH = T // 2
        NTH = NT // 2
        rings3 = [nc.sync, nc.scalar, nc.gpsimd]
        x_engs = cycle(rings3)
        for h in range(2):
            for g in range(G):
                hsl = slice(h * TH, (h + 1) * TH)
                next(x_engs).dma_start(
                    out=x_cur[g][:, hsl], in_=x_ext[g * 128:(g + 1) * 128, hsl])

        fc_dma(0, 1)
        fc_dma(0, 2)

        for h in range(2):
            for g in range(G):
                for sub in range(NTH):
                    tj = h * NTH + sub
                    csl = slice(tj * TCH, (tj + 1) * TCH)
                    nc.vector.tensor_reduce(
                        xsum_cur[:, g, tj:tj + 1], x_cur[g][:, csl],
                        axis=AX.X, op=OP.add)
                    emit_sq_chunk(x_cur[g][:, csl], xsq_cur[:, g, tj:tj + 1],
                                  parity=1, tag=f"xin_{g}_{tj}")

        st1 = wprep_state()
        wprep_part(0, 1, st1, 0, KW * G, rings=rings3)

        # small per-channel vectors now (gpsimd ring is free again)
        alpha_t = {1: load_pcvec3("alpha1_all", alpha1_ext),
                   2: load_pcvec3("alpha2_all", alpha2_ext)}
        g_t = {1: load_pcvec3("g1_all", conv1_g_ext),
               2: load_pcvec3("g2_all", conv2_g_ext)}
        cb_t = {1: load_pcvec3("cb1_all", conv1_b_ext),
                2: load_pcvec3("cb2_all", conv2_b_ext)}
        for which, bext in ((1, fc1_b_ext), (2, fc2_b_ext)):
            t = persist.tile([128, 3 * NCC], f32, name=f"fcb{which}_all")
            nc.gpsimd.dma_start(
                out=t, in_=bext.rearrange("i (c p) -> p (i c)", p=128))
            fcb_all[which] = t
        for which in (1, 2):
            allt = persist.tile([128, 3 * G], f32, name=f"sqS{which}_all")
            emit_rsqrt(allt, alpha_t[which][:, 0:3 * G], f"sa{which}")
            sqS_t[which] = allt
            inv = persist.tile([128, 3 * G], f32, name=f"invA{which}_all")
            nc.vector.reciprocal(inv, alpha_t[which][:, 0:3 * G])
            invA_t[which] = inv

        # layer-0 fc + style coefficients + conv1 weight norm: all cheap work
        # that overlaps the x-load tail / W-gated conv trickle
        hcoef = {}
        for which in (1, 2):
            h = fc_mm(0, which)
            hcoef[(0, which)] = emit_coef(f"l0_{which}", h,
                                          lay(alpha_t[which], 0))
        W1_cur, wnS1_cur = wprep_norm(0, 1, st1)
        W2_cur, wnS2_cur = None, None  # prepped during conv1_0

        # ------------- iterations -------------
        pending_bias = None

        for i in range(n_iters):
            d = DILATIONS[i]
            coef1 = hcoef.pop((i, 1))
            coef2 = hcoef.pop((i, 2))

            sinS1, sinB1 = emit_stats(f"a1_{i}", xsum_cur, xsq_cur, coef1)

            def snake1(cj, i=i, sS=sinS1, sB=sinB1):
                snake_chunk(f"s1_{i}", cj,
                            src_fn=lambda g: x_cur[g][:, 0:T],
                            dst_fn=lambda g: b1pad[g][:, PADL:PADL + T],
                            sinS=sS, sinB=sB,
                            sqS=lay(sqS_t[1], i), invA=lay(invA_t[1], i))

            c1sum = small.tile([128, G, NT], f32, tag="c1sum", name=f"c1sum_{i}")
            c1sq = small.tile([128, G, NT], f32, tag="c1sq", name=f"c1sq_{i}")

            # conv1 bias is a per-channel constant absorbed exactly by the
            # following instance norm, so it is not applied at all.
            def evict1(co, tj, pt, c1sum=c1sum, wnS1=wnS1_cur):
                dst = cb2pad[co][:, 1 + tj * TCH: 1 + (tj + 1) * TCH]
                if (tj + co) % 2 == 0:
                    nc.scalar.activation(
                        out=dst, in_=pt, func=AF.Identity,
                        bias=zero_col, scale=wnS1[:, co:co + 1],
                        accum_out=c1sum[:, co, tj:tj + 1])
                else:
                    nc.vector.tensor_scalar(
                        dst, pt, wnS1[:, co:co + 1], None, OP.mult, OP.add,
                        accum_out=c1sum[:, co, tj:tj + 1])

            # squares straight from PSUM, concurrent with the eviction (both
            # only read pt); the Square's per-partition scale folds wnS in,
            # so the accumulated slots are sums of the SCALED output squares
            def post1(co, tj, pt, i=i, c1sq=c1sq, wnS1=wnS1_cur):
                dst = scr.tile([128, TCH], bf16, tag="sqd",
                               name=f"sqd_c1_{i}_{co}_{tj}", bufs=3)
                nc.scalar.activation(out=dst, in_=pt, func=AF.Square,
                                     scale=wnS1[:, co:co + 1],
                                     bias=zero_col,
                                     accum_out=c1sq[:, co, tj:tj + 1])

            # prefetch conv2's weights + next layer's fc weights mid-conv1
            st2 = wprep_state()
            inject1 = {
                1: [lambda st2=st2, i=i: wprep_part(i, 2, st2, 0, 4)],
                2: [lambda st2=st2, i=i: wprep_part(i, 2, st2, 4, 8)],
                3: [lambda st2=st2, i=i: wprep_part(i, 2, st2, 8, 12)],
            }
            if i < n_iters - 1:
                inject1[4] = [lambda i=i: (fc_dma(i + 1, 1), fc_dma(i + 1, 2))]

            emit_conv(f"c1_{i}", W1_cur, b1pad, PADL, d, evict1, post1,
                      snake_fn=snake1, inject=inject1)

            # norm matmuls for conv2's weights: PE filler in the stats bubble
            W2_cur, wnS2_cur = wprep_norm(i, 2, st2)

            sinS2, sinB2 = emit_stats(f"a2_{i}", c1sum, c1sq, coef2)

            def snake2(cj, i=i, sS=sinS2, sB=sinB2):
                snake_chunk(f"s2_{i}", cj,
                            src_fn=lambda g: cb2pad[g][:, 1:1 + T],
                            dst_fn=lambda g: cb2pad[g][:, 1:1 + T],
                            sinS=sS, sinB=sB,
                            sqS=lay(sqS_t[2], i), invA=lay(invA_t[2], i))

            # conv2 bias is a per-channel constant: every downstream consumer
            # of x except the final output is an instance norm (which absorbs
            # it) or the residual chain. Accumulate it and apply per chunk
            # just before the final conv2's evictions.
            if pending_bias is None:
                pending_bias = small.tile([128, G], f32, tag="pend",
                                          name="pending_bias", bufs=1)
                nc.vector.tensor_copy(pending_bias, lay(cb_t[2], i))
            else:
                nc.vector.tensor_tensor(out=pending_bias, in0=pending_bias,
                                        in1=lay(cb_t[2], i), op=OP.add)

            last = (i == n_iters - 1)

            def bias_wave(wave, pending_bias=pending_bias):
                # per-chunk deferred-bias adds for this wave's chunks, emitted
                # after the wave's snake chunks so they don't block the snake
                # Sin ops on the in-order ACT queue
                for co in range(G):
                    for tj in wave:
                        sl = x_cur[co][:, tj * TCH:(tj + 1) * TCH]
                        nc.scalar.activation(
                            out=sl, in_=sl, func=AF.Identity,
                            bias=pending_bias[:, co:co + 1], scale=1.0)

            xsum_nxt = small.tile([128, G, NT], f32, tag="xsum", name=f"xsum_{i}")
            xsq_nxt = small.tile([128, G, NT], f32, tag="xsq", name=f"xsq_{i}")

            def evict2(co, tj, pt, wnS2=wnS2_cur, xsum_nxt=xsum_nxt):
                sl = x_cur[co][:, tj * TCH:(tj + 1) * TCH]
                nc.vector.scalar_tensor_tensor(
                    out=sl, in0=pt, scalar=wnS2[:, co:co + 1], in1=sl,
                    op0=OP.mult, op1=OP.add,
                    accum_out=xsum_nxt[:, co, tj:tj + 1])

            def post2(co, tj, pt, i=i, xsq_nxt=xsq_nxt, last=last):
                sl = x_cur[co][:, tj * TCH:(tj + 1) * TCH]
                if last:
                    # stream the finished chunk straight out
                    hw_eng().dma_start(
                        out=out_ext[co * 128:(co + 1) * 128,
                                    tj * TCH:(tj + 1) * TCH],
                        in_=sl)
                else:
                    emit_sq_chunk(sl, xsq_nxt[:, co, tj:tj + 1],
                                  parity=(tj + co) % 2, tag=f"x_{i}_{co}_{tj}")

            # prefetch next layer's conv1 weights + fc matmuls mid-conv2
            inject2 = {}
            st1n = wprep_state()
            if i < n_iters - 1:
                def fc_next(i=i):
                    for which in (1, 2):
                        h = fc_mm(i + 1, which)
                        hcoef[(i + 1, which)] = emit_coef(
                            f"l{i + 1}_{which}", h,
                            lay(alpha_t[which], i + 1))
                inject2 = {
                    1: [lambda st1n=st1n, i=i: wprep_part(i + 1, 1, st1n, 0, 4)],
                    2: [lambda st1n=st1n, i=i: wprep_part(i + 1, 1, st1n, 4, 8)],
                    3: [lambda st1n=st1n, i=i: wprep_part(i + 1, 1, st1n, 8, 12)],
                    4: [fc_next],
                }

            emit_conv(f"c2_{i}", W2_cur, cb2pad, 1, 1, evict2, post2,
                      snake_fn=snake2, inject=inject2,
                      pre_fn=bias_wave if last else None)
            xsum_cur, xsq_cur = xsum_nxt, xsq_nxt

            if i < n_iters - 1:
                # norm matmuls: PE filler in the next stats bubble
                W1_cur, wnS1_cur = wprep_norm(i + 1, 1, st1n)

    return nc


def make_in_maps(inputs, T=T_FULL):
    npf = lambda v: np.asarray(v, dtype=np.float32)
    x = npf(inputs["x"])
    s = npf(inputs["s"])
    shared = {
        "fc1_w": npf(inputs["fc1_w"]),
        "fc1_b": npf(inputs["fc1_b"]),
        "alpha1": npf(inputs["alpha1"]).reshape(3, C),
        "conv1_v": npf(inputs["conv1_v"]),
        "conv1_g": npf(inputs["conv1_g"]),
        "conv1_b": npf(inputs["conv1_b"]),
        "fc2_w": npf(inputs["fc2_w"]),
        "fc2_b": npf(inputs["fc2_b"]),
        "alpha2": npf(inputs["alpha2"]).reshape(3, C),
        "conv2_v": npf(inputs["conv2_v"]),
        "conv2_g": npf(inputs["conv2_g"]),
        "conv2_b": npf(inputs["conv2_b"]),
    }
    in_maps = []
    for b in range(N_CORES):
        m = dict(shared)
        m["x"] = np.ascontiguousarray(x[b, :T, :].T)
        m["s"] = np.ascontiguousarray(s[b].reshape(S, 1))
        in_maps.append(m)
    return in_maps


_CACHED = {}


def kernel(**inputs) -> np.ndarray:
    from concourse.bass_utils import run_bass_kernel_spmd

    max_alpha = float(max(np.abs(np.asarray(inputs["alpha1"])).max(),
                          np.abs(np.asarray(inputs["alpha2"])).max()))
    key = ("nc", max_alpha)
    if key not in _CACHED:
        nc = build_nc(T_FULL, max_alpha=max_alpha)
        nc.finalize()
        _CACHED[key] = nc
    nc = _CACHED[key]
    in_maps = make_in_maps(inputs, T_FULL)
    res = run_bass_kernel_spmd(nc, in_maps, core_ids=list(range(N_CORES)))
    out = np.stack(
        [np.asarray(res.results[i]["out"]).T for i in range(N_CORES)], axis=0)
    return np.ascontiguousarray(out).astype(np.float32)
